# revision 2
# baseline (speedup 1.0000x reference)
"""Trainium2 Bass kernel for DeformableConvBlock (B=4, C=64, H=W=128, K=3).

Self-contained: builds an SPMD Bass/Tile program for 8 NeuronCores.
Core c handles image c//2, output-row half c%2 (data-parallel over
batch x row-halves). Per core: offset conv on the PE (9 shifted
accumulating matmuls), DMA-transpose of offsets to pixel-major,
bilinear weight/index prep on DVE+Scalar (positions clamped into a
zero-padded table so no validity masking is needed), per-(tap,pixel)
2x2-patch gathers from a plane-major patch table via GPSIMD SWDGE
dma_gather, contiguous bilinear reduction on the DVE, PE-transposes
(is_transpose matmuls, keeping the DMA rings free for gathers) of the
sampled tensor to contraction-major, and the 576-contraction conv
matmul on the PE.

kernel(**inputs) takes the full unsharded numpy inputs and returns the
full [4, 64, 128, 128] float32 output.
"""
from contextlib import ExitStack

import numpy as np
import ml_dtypes

import concourse.bacc as bacc
import concourse.bass as bass
import concourse.mybir as mybir
import concourse.tile as tile
from concourse.tile import TileContext
from concourse.vector_clock import ScopedClock, VectorClock

F32 = mybir.dt.float32
BF16 = mybir.dt.bfloat16
I32 = mybir.dt.int32
I16 = mybir.dt.int16
AF = mybir.ActivationFunctionType
OP = mybir.AluOpType

H = W = 128
C = 64
O = 64
KK = 9
ROWS = 64            # output rows per core
NPX = ROWS * W       # 8192
PADT = 4             # table padding on each side
PW = W + 2 * PADT    # 136
NTAB = PW * PW       # 18496 table rows
EROW = 4 * C         # 256 elems per table row (4 corner planes x 64c)
KC = KK * C          # 576 contraction size
JT = 5               # 128-row contraction tiles (4 full + 1 of 64)
GROUP = 4            # rows per gather group
SUPER = 8            # rows per transpose batch
BIG = 1023.5         # round-to-nearest floor offset (HW rounds converts)
CLO = -3.96875       # position clamp (offsets are within +-2.8)
CHI = 130.96875


class TileContextSplitDrain(TileContext):
    """Stock epilogue emits one Drain with one wait per outstanding proc;
    this walrus rejects >1 sync wait per instruction, so emit one Drain
    per proc instead."""

    def _drain_and_barrier(self, tick_clock, wait_clock):
        gc = tick_clock.global_clock
        nprocs = len(gc)
        emitted = False
        for p in range(nprocs):
            t = gc[p]
            if t <= 0:
                continue
            vec = [0] * nprocs
            vec[p] = t
            drain_inst = self.nc.sync.drain()
            wait_clock.add_sem_waits(
                drain_inst.ins, ScopedClock({None: VectorClock(vec)})
            )
            si = drain_inst.ins.sync_info
            assert si is None or len(si.on_wait) <= 1
            emitted = True
        if not emitted:
            self.nc.sync.drain()
        self.nc.all_engine_barrier()
        assert self.sems is not None
        popped = self.nc._tile_sem_poison_stack.pop()
        assert popped is self._sem_poison
        self.nc.clear_and_free_semaphores(list(self.sems.allocated().values()))
        self.nc.all_engine_barrier()


def build_program(nrows=ROWS):
    """Build the SPMD Bass program. nrows<=64 shrinks work for sim tests."""
    npx = nrows * W
    n16 = max(1, npx // 512)          # 512-px chunks for offset conv
    ngroups = nrows // GROUP
    nsupers = max(1, nrows // SUPER)

    nc = bacc.Bacc(num_swdge_queues=4)
    xband = nc.dram_tensor("xband", [C, nrows + 2, W + 2], BF16, kind="ExternalInput")
    table = nc.dram_tensor("table", [NTAB, EROW], BF16, kind="ExternalInput")
    w_off = nc.dram_tensor("w_off", [KK, C, 18], BF16, kind="ExternalInput")
    b_off = nc.dram_tensor("b_off", [18, 1], F32, kind="ExternalInput")
    w2 = nc.dram_tensor("w2", [JT, 128, O], BF16, kind="ExternalInput")
    b2 = nc.dram_tensor("b2", [O, 1], F32, kind="ExternalInput")
    cgrid = nc.dram_tensor("cgrid", [1, nrows], F32, kind="ExternalInput")
    iotax = nc.dram_tensor("iotax", [128, 1], F32, kind="ExternalInput")
    ident = nc.dram_tensor("ident", [128, 128], BF16, kind="ExternalInput")
    out = nc.dram_tensor("out", [O, npx], F32, kind="ExternalOutput")

    ctx = ExitStack()
    with TileContextSplitDrain(nc) as tc:
        const_pool = ctx.enter_context(tc.tile_pool(name="const", bufs=1))
        big_pool = ctx.enter_context(tc.tile_pool(name="big", bufs=1))
        prep_pool = ctx.enter_context(tc.tile_pool(name="prep", bufs=1))
        g_pool = ctx.enter_context(tc.tile_pool(name="g", bufs=2))
        t1_pool = ctx.enter_context(tc.tile_pool(name="t1", bufs=2))
        s_pool = ctx.enter_context(tc.tile_pool(name="s", bufs=2))
        st_pool = ctx.enter_context(tc.tile_pool(name="st", bufs=2))
        o_pool = ctx.enter_context(tc.tile_pool(name="o", bufs=2))
        psum_pool = ctx.enter_context(tc.tile_pool(name="ps", bufs=2, space="PSUM"))
        psumt_pool = ctx.enter_context(tc.tile_pool(name="pst", bufs=4, space="PSUM"))
        psum2_pool = ctx.enter_context(tc.tile_pool(name="ps2", bufs=2, space="PSUM"))

        # ---- loads ----
        xb = const_pool.tile([C, (nrows + 2) * (W + 2)], BF16)
        nc.sync.dma_start(out=xb[:], in_=xband[:].rearrange("c h w -> c (h w)"))
        xb_v = xb[:].rearrange("c (h w) -> c h w", h=nrows + 2, w=W + 2)

        wof = const_pool.tile([C, KK * 18], BF16)
        wof_v = wof[:].rearrange("c (k e) -> c k e", k=KK, e=18)
        nc.sync.dma_start(out=wof_v, in_=w_off[:].rearrange("k c e -> c k e"))

        bof = const_pool.tile([18, 1], F32)
        nc.sync.dma_start(out=bof[:], in_=b_off[:])

        w2t = const_pool.tile([128, JT * O], BF16)
        w2t_v = w2t[:].rearrange("p (j e) -> p j e", j=JT, e=O)
        nc.sync.dma_start(out=w2t_v, in_=w2[:].rearrange("j p e -> p j e"))

        b2t = const_pool.tile([O, 1], F32)
        nc.sync.dma_start(out=b2t[:], in_=b2[:])

        idt = const_pool.tile([128, 128], BF16)
        nc.sync.dma_start(out=idt[:], in_=ident[:])

        # broadcast const grids across partitions
        cy = const_pool.tile([128, nrows], F32)
        nc.sync.dma_start(out=cy[:], in_=cgrid[0:1, :].to_broadcast((128, nrows)))
        iox = const_pool.tile([128, 1], F32)
        nc.sync.dma_start(out=iox[:], in_=iotax[:])

        # ---- stage 1: offset conv ----
        oc = big_pool.tile([32, npx], BF16)
        nc.gpsimd.memset(oc[:], 0.0)
        for t in range(n16):
            ps = psum_pool.tile([18, 512], F32, tag="ps1")
            r0 = t * 4  # first output row in chunk
            for k in range(KK):
                dy, dx = k // 3, k % 3
                rhs = xb_v[:, r0 + dy:r0 + dy + 4, dx:dx + W]
                nc.tensor.matmul(
                    out=ps[:], lhsT=wof_v[:, k, :], rhs=rhs,
                    start=(k == 0), stop=(k == KK - 1),
                )
            nc.scalar.activation(
                out=oc[:18, t * 512:(t + 1) * 512], in_=ps[:],
                func=AF.Identity, bias=bof[:],
            )

        # ---- stage 2: offsets to pixel-major ----
        op = big_pool.tile([128, nrows * 32], BF16)
        op3 = op[:].rearrange("p (y e) -> p y e", y=nrows, e=32)
        nc.sync.dma_start_transpose(out=op3, in_=oc[:])

        # ---- stage 3: wf/idx prep (all rows at once) ----
        # positions clamped into the zero-padded table => no validity ops;
        # floor via round-to-nearest convert of (s + 1023.5).
        _ppn = [0]

        def pp(dt=F32):
            _ppn[0] += 1
            return prep_pool.tile([128, nrows * KK], dt, tag=f"prep{_ppn[0]}", name=f"prep{_ppn[0]}")

        opf = prep_pool.tile([128, nrows * 18], F32, tag="opf")
        opf_v = opf[:].rearrange("p (y e) -> p y e", y=nrows, e=18)
        nc.vector.tensor_copy(out=opf_v, in_=op3[:, :, 0:18])

        wf = big_pool.tile([128, nrows * 36], BF16)
        wf_v = wf[:].rearrange("p (y k c) -> p y k c", y=nrows, k=KK, c=4)
        idx = big_pool.tile([128, nrows * KK], I16)

        fls = {}
        for axis in (0, 1):  # 0: y, 1: x
            s = pp()
            s3 = s[:].rearrange("p (y k) -> p y k", y=nrows, k=KK)
            off_src = opf_v[:, :, axis * 9:axis * 9 + 9]
            if axis == 0:
                # sy = off_y + (ky-1 folded into bias) + row grid
                cy3 = cy[:, :, None].to_broadcast((128, nrows, KK))
                nc.vector.tensor_tensor(out=s3, in0=off_src, in1=cy3, op=OP.add)
            else:
                # sx = off_x + (kx-1 folded into bias) + column (partition)
                nc.scalar.activation(out=s3, in_=off_src, func=AF.Identity,
                                     bias=iox[:])
            # clamp into padded-table range
            nc.vector.tensor_scalar(
                out=s[:], in0=s[:], scalar1=CLO, scalar2=CHI,
                op0=OP.max, op1=OP.min)
            # floor: i0 = round(s + 1023.5); f0 = i0 - 1024
            i0 = pp(I32)
            nc.vector.tensor_scalar_add(out=i0[:], in0=s[:], scalar1=BIG)
            f0 = pp()
            nc.vector.tensor_scalar_add(out=f0[:], in0=i0[:], scalar1=-1024.0)
            # fractions
            w1 = pp()
            nc.vector.tensor_tensor(out=w1[:], in0=s[:], in1=f0[:], op=OP.subtract)
            w0 = pp()
            nc.scalar.activation(out=w0[:], in_=w1[:], func=AF.Identity,
                                 scale=-1.0, bias=1.0)
            fls[axis] = (f0, w0, w1)

        y_f0, y_w0, y_w1 = fls[0]
        x_f0, x_w0, x_w1 = fls[1]

        # wf[...,(i,j)] = wy_i * wx_j   (corner planes 00,01,10,11)
        for i, wy in enumerate((y_w0, y_w1)):
            for j, wx in enumerate((x_w0, x_w1)):
                dst = wf_v[:, :, :, 2 * i + j].rearrange("p y k -> p (y k)")
                nc.vector.tensor_tensor(out=dst, in0=wy[:], in1=wx[:], op=OP.mult)

        # idx = (y0+4)*136 + (x0+4)
        idxf = pp()
        nc.vector.tensor_scalar(
            out=idxf[:], in0=y_f0[:], scalar1=float(PW), scalar2=float(PADT * PW + PADT),
            op0=OP.mult, op1=OP.add)
        nc.vector.tensor_tensor(out=idxf[:], in0=idxf[:], in1=x_f0[:], op=OP.add)
        nc.vector.tensor_copy(out=idx[:], in_=idxf[:])
        # wrapped-16 index layout for dma_gather queue 0 (cores 0/1 read
        # partitions 0-15 / 16-31): wrapped[r*16+p16, yk*8+q] = idx[q*16+p16, yk]
        # for r in {0,1}. Built with partition stream-shuffles, no DMA.
        nyk = nrows * KK
        wrapped = big_pool.tile([128, nyk * 8], I16)
        nc.gpsimd.memset(wrapped[:], 0)
        wr_v = wrapped[:].rearrange("p (yk q) -> p yk q", yk=nyk, q=8)
        for qj in range(4):
            for qh in range(2):
                mask = [16 * qh + (p % 16) for p in range(32)]
                nc.vector.stream_shuffle(
                    out=wr_v[0:32, :, 2 * qj + qh],
                    in_=idx[32 * qj:32 * (qj + 1), :], mask=mask)
        for qt in range(1, 4):
            nc.vector.tensor_copy(
                out=wrapped[32 * qt:32 * (qt + 1), :], in_=wrapped[0:32, :])

        # ---- stages 4-6 ----
        for sg in range(nsupers):
            s8 = s_pool.tile([128, SUPER * KC], BF16, tag="s8")
            s8_v = s8[:].rearrange("p (y e) -> p y e", y=SUPER, e=KC)
            for gi in range(SUPER // GROUP):
                y0 = sg * SUPER + gi * GROUP
                g = g_pool.tile([128, GROUP * KK * EROW], BF16, tag="g")
                g_m = g[:].rearrange("p (m e) -> p m e", m=GROUP * KK, e=EROW)
                nidx_g = 128 * GROUP * KK
                nc.gpsimd.dma_gather(
                    out_ap=g_m, in_ap=table[:],
                    idxs_ap=wrapped[:, y0 * KK * 8:(y0 + GROUP) * KK * 8],
                    num_idxs=nidx_g, num_idxs_reg=nidx_g, elem_size=EROW,
                    single_packet=False, queue_num=(y0 // GROUP) % 4)
                # weighted corners: planes are contiguous 64-blocks
                g_v = g[:].rearrange(
                    "p (m j c) -> p m j c", m=GROUP * KK, j=4, c=C)
                wfb = wf_v[:, y0:y0 + GROUP, :, :].rearrange(
                    "p y k j -> p (y k) j")[:, :, :, None].to_broadcast(
                    (128, GROUP * KK, 4, C))
                nc.vector.tensor_tensor(out=g_v, in0=g_v, in1=wfb, op=OP.mult)
                t1 = t1_pool.tile([128, GROUP * KK * C * 2], BF16, tag="t1")
                t1_v = t1[:].rearrange(
                    "p (m j c) -> p m j c", m=GROUP * KK, j=2, c=C)
                nc.vector.tensor_tensor(
                    out=t1_v, in0=g_v[:, :, 0:2, :], in1=g_v[:, :, 2:4, :],
                    op=OP.add)
                sdst = s8_v[:, gi * GROUP:(gi + 1) * GROUP, :].rearrange(
                    "p y (k c) -> p (y k) c", k=KK, c=C)
                nc.vector.tensor_tensor(
                    out=sdst, in0=t1_v[:, :, 0, :], in1=t1_v[:, :, 1, :],
                    op=OP.add)

            # PE-transpose s8 -> st (contraction-major), via PSUM bf16
            st = st_pool.tile([128, JT * SUPER * 128], BF16, tag="st")
            st_v = st[:].rearrange("p (j y c) -> p j y c", j=JT, y=SUPER, c=128)
            for j in range(JT):
                jw = min(128, KC - j * 128)
                pst = psumt_pool.tile([128, SUPER * 128], BF16, tag="pst")
                for y in range(SUPER):
                    nc.tensor.matmul(
                        out=pst[0:jw, y * 128:(y + 1) * 128],
                        lhsT=s8_v[:, y, j * 128:j * 128 + jw],
                        rhs=idt[:], is_transpose=True,
                        start=True, stop=True, skip_group_check=True,
                    )
                nc.scalar.activation(
                    out=st_v[0:jw, j, :, :], in_=pst[0:jw, :],
                    func=AF.Identity)

            for half in range(SUPER * 128 // 512):
                ps2 = psum2_pool.tile([O, 512], F32, tag="ps2")
                for j in range(JT):
                    jw = min(128, KC - j * 128)
                    rhs = st_v[0:jw, j, 4 * half:4 * half + 4, :]
                    nc.tensor.matmul(
                        out=ps2[:], lhsT=w2t_v[0:jw, j, :], rhs=rhs,
                        start=(j == 0), stop=(j == JT - 1),
                    )
                ob = o_pool.tile([O, 512], F32, tag="ob")
                nc.scalar.activation(
                    out=ob[:], in_=ps2[:], func=AF.Identity, bias=b2t[:])
                pc = sg * (SUPER * 128 // 512) + half
                nc.sync.dma_start(out=out[:, pc * 512:(pc + 1) * 512], in_=ob[:])
        ctx.close()
    nc.compile()
    return nc


# ---------------- host side ----------------

def host_prepare(x, off_w, off_b, weight, bias, nrows=ROWS):
    """Build per-core input maps."""
    B = x.shape[0]
    x = np.asarray(x, np.float32)
    # padded image for offset conv, bf16, [B, C, H+2, W+2]
    xpad = np.pad(x, ((0, 0), (0, 0), (1, 1), (1, 1))).astype(ml_dtypes.bfloat16)
    # patch table per image: padded-by-PADT, channels-last, corner-plane-major
    xp2 = np.pad(x, ((0, 0), (0, 0), (PADT, PADT + 1), (PADT, PADT + 1)))
    xcl = xp2.transpose(0, 2, 3, 1)        # [B, PW+1, PW+1, C]
    tables = []
    for b in range(B):
        t = np.empty((PW, PW, 4, C), np.float32)
        t[:, :, 0] = xcl[b, :PW, :PW]
        t[:, :, 1] = xcl[b, :PW, 1:PW + 1]
        t[:, :, 2] = xcl[b, 1:PW + 1, :PW]
        t[:, :, 3] = xcl[b, 1:PW + 1, 1:PW + 1]
        tables.append(t.reshape(NTAB, EROW).astype(ml_dtypes.bfloat16))

    # offset conv weights: channel perm [dy taps 0..8, dx taps 0..8]
    perm = [2 * k for k in range(KK)] + [2 * k + 1 for k in range(KK)]
    w_off_p = np.asarray(off_w, np.float32)[perm]          # [18, C, 3, 3]
    # lhsT per tap: tap k = dy*3+dx -> [C, 18]
    w_off_t = np.empty((KK, C, 18), np.float32)
    for k in range(KK):
        dy, dx = k // 3, k % 3
        w_off_t[k] = w_off_p[:, :, dy, dx].T               # [C, 18]
    w_off_t = w_off_t.astype(ml_dtypes.bfloat16)
    b_off_p = np.asarray(off_b, np.float32)[perm].reshape(18, 1).copy()
    # fold the kernel-tap grid shift (ky-1 / kx-1) into the conv bias
    for k in range(KK):
        b_off_p[k, 0] += float(k // 3 - 1)
        b_off_p[9 + k, 0] += float(k % 3 - 1)

    # main weights: W2[(k,c), o] = weight[o, c, k], padded to 640 rows
    wgt = np.asarray(weight, np.float32).reshape(O, C, KK)
    w2f = np.zeros((JT * 128, O), np.float32)
    kc = wgt.transpose(2, 1, 0).reshape(KK * C, O)          # [(k,c), O]
    w2f[:KK * C] = kc
    w2f = w2f.reshape(JT, 128, O).astype(ml_dtypes.bfloat16)
    b2f = np.asarray(bias, np.float32).reshape(O, 1)

    iotax = np.arange(128, dtype=np.float32).reshape(128, 1)
    identm = np.eye(128, dtype=np.float32).astype(ml_dtypes.bfloat16)

    in_maps = []
    for core in range(8):
        b, hh = core // 2, core % 2
        y0 = hh * 64
        cgrid = np.arange(y0, y0 + nrows, dtype=np.float32).reshape(1, nrows)
        in_maps.append({
            "xband": np.ascontiguousarray(xpad[b, :, y0:y0 + nrows + 2, :]),
            "table": tables[b],
            "w_off": w_off_t,
            "b_off": b_off_p,
            "w2": w2f,
            "b2": b2f,
            "cgrid": cgrid,
            "iotax": iotax,
            "ident": identm,
        })
    return in_maps


def assemble(outs, nrows=ROWS):
    """outs: list of 8 dicts with 'out' [O, nrows*W] -> [4, O, H, W]"""
    full = np.zeros((4, O, H, W), np.float32)
    for core, om in enumerate(outs):
        b, hh = core // 2, core % 2
        full[b, :, hh * 64:hh * 64 + nrows] = om["out"].reshape(O, nrows, W)
    return full


_CACHE = {}


def kernel(x, off_w, off_b, weight, bias):
    if "nc" not in _CACHE:
        _CACHE["nc"] = build_program()
    nc = _CACHE["nc"]
    in_maps = host_prepare(x, off_w, off_b, weight, bias)
    from concourse.bass_utils import run_bass_kernel_spmd
    res = run_bass_kernel_spmd(nc, in_maps, core_ids=list(range(8)))
    return assemble(res.results)


# revision 7
# speedup vs baseline: 1.4733x; 1.4733x over previous
"""Trainium2 Bass kernel for DeformableConvBlock (B=4, C=64, H=W=128, K=3).

Self-contained: builds an SPMD Bass/Tile program for 8 NeuronCores.
Core c handles image c//2, output-row half c%2 (data-parallel over
batch x row-halves). Per core: offset conv on the PE (9 shifted
accumulating matmuls), DMA-transpose of offsets to pixel-major,
bilinear weight/index prep on DVE+Scalar (positions clamped into a
zero-padded table so no validity masking is needed), per-(tap,pixel)
2x2-patch gathers from a plane-major patch table via GPSIMD SWDGE
dma_gather, contiguous bilinear reduction on the DVE, PE-transposes
(is_transpose matmuls, keeping the DMA rings free for gathers) of the
sampled tensor to contraction-major, and the 576-contraction conv
matmul on the PE.

kernel(**inputs) takes the full unsharded numpy inputs and returns the
full [4, 64, 128, 128] float32 output.
"""
from contextlib import ExitStack

import numpy as np
import ml_dtypes

import concourse.bacc as bacc
import concourse.bass as bass
import concourse.mybir as mybir
import concourse.tile as tile
from concourse.tile import TileContext
from concourse.vector_clock import ScopedClock, VectorClock

F32 = mybir.dt.float32
BF16 = mybir.dt.bfloat16
I32 = mybir.dt.int32
I16 = mybir.dt.int16
AF = mybir.ActivationFunctionType
OP = mybir.AluOpType

H = W = 128
C = 64
O = 64
KK = 9
ROWS = 64            # output rows per core
NPX = ROWS * W       # 8192
PADT = 4             # table padding on each side
PW = W + 2 * PADT    # 136
NTAB = PW * PW       # 18496 table rows
EROW = 4 * C         # 256 elems per table row (4 corner planes x 64c)
KC = KK * C          # 576 contraction size
JT = 5               # 128-row contraction tiles (4 full + 1 of 64)
GROUP = 4            # rows per gather group
SUPER = 8            # rows per transpose batch
BIG = 1023.5         # round-to-nearest floor offset (HW rounds converts)
CLO = -3.96875       # position clamp (offsets are within +-2.8)
CHI = 130.96875


class TileContextSplitDrain(TileContext):
    """Stock epilogue emits one Drain with one wait per outstanding proc;
    this walrus rejects >1 sync wait per instruction, so emit one Drain
    per proc instead."""

    def _drain_and_barrier(self, tick_clock, wait_clock):
        gc = tick_clock.global_clock
        nprocs = len(gc)
        emitted = False
        for p in range(nprocs):
            t = gc[p]
            if t <= 0:
                continue
            vec = [0] * nprocs
            vec[p] = t
            drain_inst = self.nc.sync.drain()
            wait_clock.add_sem_waits(
                drain_inst.ins, ScopedClock({None: VectorClock(vec)})
            )
            si = drain_inst.ins.sync_info
            assert si is None or len(si.on_wait) <= 1
            emitted = True
        if not emitted:
            self.nc.sync.drain()
        self.nc.all_engine_barrier()
        assert self.sems is not None
        popped = self.nc._tile_sem_poison_stack.pop()
        assert popped is self._sem_poison
        self.nc.clear_and_free_semaphores(list(self.sems.allocated().values()))
        self.nc.all_engine_barrier()


def build_program(nrows=ROWS):
    """Build the SPMD Bass program. nrows<=64 shrinks work for sim tests."""
    npx = nrows * W
    n16 = max(1, npx // 512)          # 512-px chunks for offset conv
    ngroups = nrows // GROUP
    nsupers = max(1, nrows // SUPER)

    nc = bacc.Bacc(num_swdge_queues=4)
    xband = nc.dram_tensor("xband", [C, nrows + 2, W + 2], BF16, kind="ExternalInput")
    table = nc.dram_tensor("table", [NTAB, EROW], BF16, kind="ExternalInput")
    w_off = nc.dram_tensor("w_off", [KK, C, 18], BF16, kind="ExternalInput")
    b_off = nc.dram_tensor("b_off", [18, 1], F32, kind="ExternalInput")
    w2 = nc.dram_tensor("w2", [JT, 128, O], BF16, kind="ExternalInput")
    b2 = nc.dram_tensor("b2", [O, 1], F32, kind="ExternalInput")
    cgrid = nc.dram_tensor("cgrid", [1, nrows], F32, kind="ExternalInput")
    iotax = nc.dram_tensor("iotax", [128, 1], F32, kind="ExternalInput")
    ident = nc.dram_tensor("ident", [128, 128], BF16, kind="ExternalInput")
    out = nc.dram_tensor("out", [O, npx], F32, kind="ExternalOutput")

    ctx = ExitStack()
    with TileContextSplitDrain(nc) as tc:
        const_pool = ctx.enter_context(tc.tile_pool(name="const", bufs=1))
        big_pool = ctx.enter_context(tc.tile_pool(name="big", bufs=1))
        prep_pool = ctx.enter_context(tc.tile_pool(name="prep", bufs=1))
        g_pool = ctx.enter_context(tc.tile_pool(name="g", bufs=3))
        wfe_pool = ctx.enter_context(tc.tile_pool(name="wfe", bufs=2))
        s_pool = ctx.enter_context(tc.tile_pool(name="s", bufs=2))
        st_pool = ctx.enter_context(tc.tile_pool(name="st", bufs=2))
        o_pool = ctx.enter_context(tc.tile_pool(name="o", bufs=2))
        psum_pool = ctx.enter_context(tc.tile_pool(name="ps", bufs=2, space="PSUM"))
        psumt_pool = ctx.enter_context(tc.tile_pool(name="pst", bufs=4, space="PSUM"))
        psum2_pool = ctx.enter_context(tc.tile_pool(name="ps2", bufs=2, space="PSUM"))

        # ---- loads ----
        xb = const_pool.tile([C, (nrows + 2) * (W + 2)], BF16)
        nc.sync.dma_start(out=xb[:], in_=xband[:].rearrange("c h w -> c (h w)"))
        xb_v = xb[:].rearrange("c (h w) -> c h w", h=nrows + 2, w=W + 2)

        wof = const_pool.tile([C, KK * 18], BF16)
        wof_v = wof[:].rearrange("c (k e) -> c k e", k=KK, e=18)
        nc.sync.dma_start(out=wof_v, in_=w_off[:].rearrange("k c e -> c k e"))

        bof = const_pool.tile([18, 1], F32)
        nc.sync.dma_start(out=bof[:], in_=b_off[:])

        w2t = const_pool.tile([128, JT * O], BF16)
        w2t_v = w2t[:].rearrange("p (j e) -> p j e", j=JT, e=O)
        nc.sync.dma_start(out=w2t_v, in_=w2[:].rearrange("j p e -> p j e"))

        b2t = const_pool.tile([O, 1], F32)
        nc.sync.dma_start(out=b2t[:], in_=b2[:])

        idt = const_pool.tile([128, 128], BF16)
        nc.sync.dma_start(out=idt[:], in_=ident[:])

        # broadcast const grids across partitions
        cy = const_pool.tile([128, nrows], F32)
        nc.sync.dma_start(out=cy[:], in_=cgrid[0:1, :].to_broadcast((128, nrows)))
        iox = const_pool.tile([128, 1], F32)
        nc.sync.dma_start(out=iox[:], in_=iotax[:])

        # ---- stage 1: offset conv ----
        oc = big_pool.tile([32, npx], BF16)
        nc.gpsimd.memset(oc[:], 0.0)
        for t in range(n16):
            ps = psum_pool.tile([18, 512], F32, tag="ps1")
            r0 = t * 4  # first output row in chunk
            for k in range(KK):
                dy, dx = k // 3, k % 3
                rhs = xb_v[:, r0 + dy:r0 + dy + 4, dx:dx + W]
                nc.tensor.matmul(
                    out=ps[:], lhsT=wof_v[:, k, :], rhs=rhs,
                    start=(k == 0), stop=(k == KK - 1),
                )
            nc.scalar.activation(
                out=oc[:18, t * 512:(t + 1) * 512], in_=ps[:],
                func=AF.Identity, bias=bof[:],
            )

        # ---- stage 2: offsets to pixel-major ----
        op = big_pool.tile([128, nrows * 32], BF16)
        op3 = op[:].rearrange("p (y e) -> p y e", y=nrows, e=32)
        nc.sync.dma_start_transpose(out=op3, in_=oc[:])

        # ---- stage 3: wf/idx prep (all rows at once) ----
        # positions clamped into the zero-padded table => no validity ops;
        # floor via round-to-nearest convert of (s + 1023.5).
        _ppn = [0]

        def pp(dt=F32):
            _ppn[0] += 1
            return prep_pool.tile([128, nrows * KK], dt, tag=f"prep{_ppn[0]}", name=f"prep{_ppn[0]}")

        opf = prep_pool.tile([128, nrows * 18], F32, tag="opf")
        opf_v = opf[:].rearrange("p (y e) -> p y e", y=nrows, e=18)
        nc.vector.tensor_copy(out=opf_v, in_=op3[:, :, 0:18])

        wf = big_pool.tile([128, nrows * 36], BF16)
        wf_v = wf[:].rearrange("p (y k c) -> p y k c", y=nrows, k=KK, c=4)
        idx = big_pool.tile([128, nrows * KK], I16)

        fls = {}
        for axis in (0, 1):  # 0: y, 1: x
            s = pp()
            s3 = s[:].rearrange("p (y k) -> p y k", y=nrows, k=KK)
            off_src = opf_v[:, :, axis * 9:axis * 9 + 9]
            if axis == 0:
                # sy = off_y + (ky-1 folded into bias) + row grid
                cy3 = cy[:, :, None].to_broadcast((128, nrows, KK))
                nc.vector.tensor_tensor(out=s3, in0=off_src, in1=cy3, op=OP.add)
            else:
                # sx = off_x + (kx-1 folded into bias) + column (partition)
                nc.scalar.activation(out=s3, in_=off_src, func=AF.Identity,
                                     bias=iox[:])
            # clamp into padded-table range
            nc.vector.tensor_scalar(
                out=s[:], in0=s[:], scalar1=CLO, scalar2=CHI,
                op0=OP.max, op1=OP.min)
            # floor: i0 = round(s + 1023.5); f0 = i0 - 1024
            i0 = pp(I32)
            nc.vector.tensor_scalar_add(out=i0[:], in0=s[:], scalar1=BIG)
            f0 = pp()
            nc.vector.tensor_scalar_add(out=f0[:], in0=i0[:], scalar1=-1024.0)
            # fractions
            w1 = pp()
            nc.vector.tensor_tensor(out=w1[:], in0=s[:], in1=f0[:], op=OP.subtract)
            w0 = pp()
            nc.scalar.activation(out=w0[:], in_=w1[:], func=AF.Identity,
                                 scale=-1.0, bias=1.0)
            fls[axis] = (f0, w0, w1)

        y_f0, y_w0, y_w1 = fls[0]
        x_f0, x_w0, x_w1 = fls[1]

        # wf[...,(i,j)] = wy_i * wx_j   (corner planes 00,01,10,11)
        for i, wy in enumerate((y_w0, y_w1)):
            for j, wx in enumerate((x_w0, x_w1)):
                dst = wf_v[:, :, :, 2 * i + j].rearrange("p y k -> p (y k)")
                nc.vector.tensor_tensor(out=dst, in0=wy[:], in1=wx[:], op=OP.mult)

        # idx = (y0+4)*136 + (x0+4)
        idxf = pp()
        nc.vector.tensor_scalar(
            out=idxf[:], in0=y_f0[:], scalar1=float(PW), scalar2=float(PADT * PW + PADT),
            op0=OP.mult, op1=OP.add)
        nc.vector.tensor_tensor(out=idxf[:], in0=idxf[:], in1=x_f0[:], op=OP.add)
        nc.vector.tensor_copy(out=idx[:], in_=idxf[:])
        # wrapped-16 index layout for dma_gather queue 0 (cores 0/1 read
        # partitions 0-15 / 16-31): wrapped[r*16+p16, yk*8+q] = idx[q*16+p16, yk]
        # for r in {0,1}. Built with partition stream-shuffles, no DMA.
        nyk = nrows * KK
        wrapped = big_pool.tile([128, nyk * 8], I16)
        nc.gpsimd.memset(wrapped[:], 0)
        wr_v = wrapped[:].rearrange("p (yk q) -> p yk q", yk=nyk, q=8)
        for qh in range(2):  # mask-outer order: avoid per-call mask reloads
            mask = [16 * qh + (p % 16) for p in range(32)]
            for qj in range(4):
                nc.vector.stream_shuffle(
                    out=wr_v[0:32, :, 2 * qj + qh],
                    in_=idx[32 * qj:32 * (qj + 1), :], mask=mask)
        for qt in range(1, 4):
            nc.vector.tensor_copy(
                out=wrapped[32 * qt:32 * (qt + 1), :], in_=wrapped[0:32, :])

        # ---- stages 4-6 ----
        for sg in range(nsupers):
            s8 = s_pool.tile([128, SUPER * KC], BF16, tag="s8")
            s8_v = s8[:].rearrange("p (y e) -> p y e", y=SUPER, e=KC)
            # expand corner weights 16-wide on the idle Scalar engine so the
            # DVE multiply sees a contiguous-16 inner dim on both operands
            wfe = wfe_pool.tile([128, SUPER * KK * 4 * 16], BF16, tag="wfe")
            wfe_v = wfe[:].rearrange(
                "p (m j e) -> p m j e", m=SUPER * KK, j=4, e=16)
            ysrc = wf_v[:, sg * SUPER:(sg + 1) * SUPER, :, :].rearrange(
                "p y k j -> p (y k) j")
            nc.scalar.activation(
                out=wfe_v, in_=ysrc[:, :, :, None].to_broadcast(
                    (128, SUPER * KK, 4, 16)), func=AF.Identity)
            wfe_g = wfe[:].rearrange(
                "p (gi m j c4 e) -> p gi m j c4 e",
                gi=SUPER // GROUP, m=GROUP * KK, j=4, c4=1, e=16)
            for gi in range(SUPER // GROUP):
                y0 = sg * SUPER + gi * GROUP
                g = g_pool.tile([128, GROUP * KK * EROW], BF16, tag="g")
                g_m = g[:].rearrange("p (m e) -> p m e", m=GROUP * KK, e=EROW)
                nidx_g = 128 * GROUP * KK
                nc.gpsimd.dma_gather(
                    out_ap=g_m, in_ap=table[:],
                    idxs_ap=wrapped[:, y0 * KK * 8:(y0 + GROUP) * KK * 8],
                    num_idxs=nidx_g, num_idxs_reg=nidx_g, elem_size=EROW,
                    single_packet=False, queue_num=(y0 // GROUP) % 4)
                # weighted corners: planes are contiguous 64-blocks
                g_v = g[:].rearrange(
                    "p (m j c4 e) -> p m j c4 e", m=GROUP * KK, j=4, c4=4, e=16)
                wfb = wfe_g[:, gi].to_broadcast((128, GROUP * KK, 4, 4, 16))
                nc.vector.tensor_tensor(out=g_v, in0=g_v, in1=wfb, op=OP.mult)
                g_v = g[:].rearrange(
                    "p (m j c) -> p m j c", m=GROUP * KK, j=4, c=C)
                # reduce corners in place inside g (planes 0:2 += planes 2:4)
                nc.vector.tensor_tensor(
                    out=g_v[:, :, 0:2, :], in0=g_v[:, :, 0:2, :],
                    in1=g_v[:, :, 2:4, :], op=OP.add)
                sdst = s8_v[:, gi * GROUP:(gi + 1) * GROUP, :].rearrange(
                    "p y (k c) -> p (y k) c", k=KK, c=C)
                nc.vector.tensor_tensor(
                    out=sdst, in0=g_v[:, :, 0, :], in1=g_v[:, :, 1, :],
                    op=OP.add)

            # PE-transpose s8 -> st (contraction-major), via PSUM bf16
            st = st_pool.tile([128, JT * SUPER * 128], BF16, tag="st")
            st_v = st[:].rearrange("p (j y c) -> p j y c", j=JT, y=SUPER, c=128)
            for j in range(JT):
                jw = min(128, KC - j * 128)
                pst = psumt_pool.tile([128, SUPER * 128], BF16, tag="pst")
                for y in range(SUPER):
                    nc.tensor.matmul(
                        out=pst[0:jw, y * 128:(y + 1) * 128],
                        lhsT=s8_v[:, y, j * 128:j * 128 + jw],
                        rhs=idt[:], is_transpose=True,
                        start=True, stop=True, skip_group_check=True,
                    )
                nc.scalar.activation(
                    out=st_v[0:jw, j, :, :], in_=pst[0:jw, :],
                    func=AF.Identity)

            for half in range(SUPER * 128 // 512):
                ps2 = psum2_pool.tile([O, 512], F32, tag="ps2")
                for j in range(JT):
                    jw = min(128, KC - j * 128)
                    rhs = st_v[0:jw, j, 4 * half:4 * half + 4, :]
                    nc.tensor.matmul(
                        out=ps2[:], lhsT=w2t_v[0:jw, j, :], rhs=rhs,
                        start=(j == 0), stop=(j == JT - 1),
                    )
                ob = o_pool.tile([O, 512], F32, tag="ob")
                nc.scalar.activation(
                    out=ob[:], in_=ps2[:], func=AF.Identity, bias=b2t[:])
                pc = sg * (SUPER * 128 // 512) + half
                nc.sync.dma_start(out=out[:, pc * 512:(pc + 1) * 512], in_=ob[:])
        ctx.close()
    nc.compile()
    return nc


# ---------------- host side ----------------

def host_prepare(x, off_w, off_b, weight, bias, nrows=ROWS):
    """Build per-core input maps."""
    B = x.shape[0]
    x = np.asarray(x, np.float32)
    # padded image for offset conv, bf16, [B, C, H+2, W+2]
    xpad = np.pad(x, ((0, 0), (0, 0), (1, 1), (1, 1))).astype(ml_dtypes.bfloat16)
    # patch table per image: padded-by-PADT, channels-last, corner-plane-major
    xp2 = np.pad(x, ((0, 0), (0, 0), (PADT, PADT + 1), (PADT, PADT + 1)))
    xcl = xp2.transpose(0, 2, 3, 1)        # [B, PW+1, PW+1, C]
    tables = []
    for b in range(B):
        t = np.empty((PW, PW, 4, C), np.float32)
        t[:, :, 0] = xcl[b, :PW, :PW]
        t[:, :, 1] = xcl[b, :PW, 1:PW + 1]
        t[:, :, 2] = xcl[b, 1:PW + 1, :PW]
        t[:, :, 3] = xcl[b, 1:PW + 1, 1:PW + 1]
        tables.append(t.reshape(NTAB, EROW).astype(ml_dtypes.bfloat16))

    # offset conv weights: channel perm [dy taps 0..8, dx taps 0..8]
    perm = [2 * k for k in range(KK)] + [2 * k + 1 for k in range(KK)]
    w_off_p = np.asarray(off_w, np.float32)[perm]          # [18, C, 3, 3]
    # lhsT per tap: tap k = dy*3+dx -> [C, 18]
    w_off_t = np.empty((KK, C, 18), np.float32)
    for k in range(KK):
        dy, dx = k // 3, k % 3
        w_off_t[k] = w_off_p[:, :, dy, dx].T               # [C, 18]
    w_off_t = w_off_t.astype(ml_dtypes.bfloat16)
    b_off_p = np.asarray(off_b, np.float32)[perm].reshape(18, 1).copy()
    # fold the kernel-tap grid shift (ky-1 / kx-1) into the conv bias
    for k in range(KK):
        b_off_p[k, 0] += float(k // 3 - 1)
        b_off_p[9 + k, 0] += float(k % 3 - 1)

    # main weights: W2[(k,c), o] = weight[o, c, k], padded to 640 rows
    wgt = np.asarray(weight, np.float32).reshape(O, C, KK)
    w2f = np.zeros((JT * 128, O), np.float32)
    kc = wgt.transpose(2, 1, 0).reshape(KK * C, O)          # [(k,c), O]
    w2f[:KK * C] = kc
    w2f = w2f.reshape(JT, 128, O).astype(ml_dtypes.bfloat16)
    b2f = np.asarray(bias, np.float32).reshape(O, 1)

    iotax = np.arange(128, dtype=np.float32).reshape(128, 1)
    identm = np.eye(128, dtype=np.float32).astype(ml_dtypes.bfloat16)

    in_maps = []
    for core in range(8):
        b, hh = core // 2, core % 2
        y0 = hh * 64
        cgrid = np.arange(y0, y0 + nrows, dtype=np.float32).reshape(1, nrows)
        in_maps.append({
            "xband": np.ascontiguousarray(xpad[b, :, y0:y0 + nrows + 2, :]),
            "table": tables[b],
            "w_off": w_off_t,
            "b_off": b_off_p,
            "w2": w2f,
            "b2": b2f,
            "cgrid": cgrid,
            "iotax": iotax,
            "ident": identm,
        })
    return in_maps


def assemble(outs, nrows=ROWS):
    """outs: list of 8 dicts with 'out' [O, nrows*W] -> [4, O, H, W]"""
    full = np.zeros((4, O, H, W), np.float32)
    for core, om in enumerate(outs):
        b, hh = core // 2, core % 2
        full[b, :, hh * 64:hh * 64 + nrows] = om["out"].reshape(O, nrows, W)
    return full


_CACHE = {}


def kernel(x, off_w, off_b, weight, bias):
    if "nc" not in _CACHE:
        _CACHE["nc"] = build_program()
    nc = _CACHE["nc"]
    in_maps = host_prepare(x, off_w, off_b, weight, bias)
    from concourse.bass_utils import run_bass_kernel_spmd
    res = run_bass_kernel_spmd(nc, in_maps, core_ids=list(range(8)))
    return assemble(res.results)
